# revision 68
# baseline (speedup 1.0000x reference)
# Trainium2 Bass kernel for GQA attention block (B=8, T=512, C=2048, 16 q heads,
# 4 kv heads, head_dim=128, RoPE, causal mask, output projection).
#
# Strategy: data parallel across the 8 NeuronCores — core i handles batch
# element i with the full weight set (no collectives). Per core everything is
# computed in a transposed layout:
#   qT/kT [d, t]  = W[:, d-tile].T-contract  (lhsT = weight tile, rhs = xT)
#   scoresT [s,t] = kT-slice.T @ qT          (softmax normalizer along the
#                                             partition dim via a ones-matmul
#                                             that broadcasts the sum to all
#                                             128 partitions for free)
#   outT [d, t]   = v-slice.T @ (mask*exp(scoresT))  (unnormalized)
#   y [t, e]      = outT-slice.T @ wo-tile   (normalized outT)
# RoPE rotate-half is a signed 128x128 permutation matmul + 2 muls + 1 add.
# Matmul operands are fp16 (full PE rate, half the HBM bytes of fp32, DVE 2x
# modes; fp32 PSUM accumulation throughout).
#
# Phase A streams v-proj and k-proj interleaved per 128-row contraction tile.
# DMA design (the early phase is supply-limited at ~250-300 GB/s aggregate):
#   - x/wq/wk/wv are pre-tiled on the host into SBUF layout so every chunk is
#     one fat contiguous descriptor per partition (256B-descriptor transfers
#     measurably starve);
#   - completion semaphores come from 8 lanes assigned round-robin over GLOBAL
#     emission order, and DMA #n's issue blocks its engine until #(n-8) fully
#     landed — the emission sequence is rotated so those waits always point at
#     short, long-finished transfers;
#   - the two JIT-critical streams (x, wv) ride the gpsimd SWDGE queue
#     (~1.5-2x the throughput of a contended HWDGE ring), except their first
#     three cts which ride the empty low-latency HWDGE rings; wk01 -> scalar,
#     wk23 -> sync;
#   - small "tick" matmuls pace cts 0-2 so the HAM clock gate never sees a
#     >3.4us PE idle (which would re-throttle to 1.2 GHz); ticks must stop
#     before the first k23 matmul (they share the av psum bank — a later
#     tick would serialize k23 behind it via tile-lifetime tracking);
#   - from ct3 the pacing is REAL k23 projection matmuls (lag-3; their
#     wk23/x operands landed groups earlier on dedicated rings).
# kv heads 0/1 catch up mid-loop so rope-k0/k1 frees the qp psum banks early;
# the k23 tail completes before stage_a(0) and rope-k2/k3 runs in the
# phase-B prologue under the first q projections (DVE/scalar FIFO order:
# after rope-q0, so neither rope-q nor the exp stream queues behind it).
# The final output-projection chain is column-split across two separate psum
# tiles so the last copy/store overlaps the last matmuls (one shared tile
# would add a tile-granular write-after-read wait). When the runtime mask is
# exactly causal, the scores/AV/normalizer matmuls restrict their moving dim
# to the visible t-range and only the diagonal 128x128 block gets
# mask-multiplied; any other mask falls back to a general masked build.

import os
import sys

import numpy as np

for _p in (
    "/root/.axon_site",
    "/root/.axon_site/_ro/trn_rl_repo",
    "/root/.axon_site/_ro/pypackages",
    "/opt/trn_rl_repo",
):
    if os.path.isdir(_p) and _p not in sys.path:
        sys.path.append(_p)

import concourse.bass as bass  # noqa: E402
import concourse.bass_isa as bass_isa  # noqa: E402
import concourse.mybir as mybir  # noqa: E402
import concourse.tile as tile  # noqa: E402
from concourse import bacc  # noqa: E402
from concourse.bass_utils import run_bass_kernel_spmd  # noqa: E402

F32 = mybir.dt.float32
F32R = mybir.dt.float32r
F16 = mybir.dt.float16
AF = mybir.ActivationFunctionType

B, T, C = 8, 512, 2048
HD, NH, NKV = 128, 16, 4
CT = C // 128  # 16 contraction tiles
TT = T // 128  # 4 t/s tiles
EG = C // 512  # 4 output column groups
REP = NH // NKV
SCALE = float(HD) ** -0.5
N_CORES = 8

_DT_ENV = os.environ.get("ATTN_DTYPE", "fp16")
MM_DT = {"fp16": F16, "fp32r": F32R, "fp32": F32}[_DT_ENV]
MM_NP = {"fp16": np.float16, "fp32r": np.float32, "fp32": np.float32}[_DT_ENV]


def _s(i, n):
    return slice(i * n, (i + 1) * n)


def _emit(tc, xT, wq, wk01, wk23, wv, wo, cosT, sinT, maskT, y, causal):
    nc = tc.nc
    mm = nc.tensor.matmul

    with (
        tc.tile_pool(name="consts", bufs=1) as consts,
        tc.tile_pool(name="work", bufs=2) as work,
        tc.tile_pool(name="ps", bufs=1, space="PSUM") as ps,
    ):
        streams = work  # one pool, one exit barrier (tags don't collide)
        cosT_sb = consts.tile([HD, T], MM_DT)
        sinT_sb = consts.tile([HD, T], MM_DT)
        # causal: one [128,128] upper-tri block reused for every diagonal
        # tile; general: the full [s,t] mask
        maskT_sb = consts.tile([128, 128] if causal else [128, TT, T], MM_DT)
        xT_sb = consts.tile([128, CT, T], MM_DT)
        kT_sb = consts.tile([HD, NKV, T], MM_DT)
        v_sb = consts.tile([128, TT, 4 * HD], MM_DT)
        aout_sb = consts.tile([HD, NH, T], MM_DT)

        # ---- PE warmup: the HAM clock gate keeps the PE at half rate until
        # a ~3.4us activity window of sustained matmuls. A short burst covers
        # the DMA-latency head (first real operands land ~1.5us after issue);
        # from there the gapless phase-A stream keeps the window busy.
        # Memsets ride the (otherwise idle) DVE so the gpsimd queue can start
        # issuing its DMA stream immediately.
        warm_sb = consts.tile([128, 256], MM_DT)
        ones_sb = consts.tile([128, 128], MM_DT)
        nc.vector.memset(warm_sb[:], 0.0)
        nc.vector.memset(ones_sb[:], 1.0)
        wps = ps.tile([HD, T], F32, tag="qp", bufs=2, name="warmps")
        for _ in range(12):
            mm(wps[:, :256], warm_sb[:, :128], warm_sb[:], start=True, stop=True)

        # ---- DMA schedule. Two hard constraints drive the emission order:
        # (1) DMA completion semaphores come from 8 lanes assigned round-robin
        #     over GLOBAL emission order — DMA #n's issue instruction blocks
        #     its engine until DMA #(n-8) has fully landed. So the first 8
        #     emitted DMAs are the urgent heads of each stream, and the
        #     rotation below keeps every lane collision pointing at a
        #     transfer that completed long before.
        # (2) wq/wk/wv are pre-tiled on the host into SBUF layout so every
        #     transfer is one fat contiguous descriptor per partition.
        # x rides the scalar HWDGE ring, wv the sync ring, wk halves the
        # gpsimd (SWDGE) ring; consts fill gpsimd's fresh-lane slots.
        wvt = {}
        wk01t = {}
        wk23t = {}

        def x_dma(c0, ncs, eng=None):
            (eng or nc.gpsimd).dma_start(
                xT_sb[:, c0 : c0 + ncs, :], xT[:, c0 * T : (c0 + ncs) * T]
            )

        def wv_dma(c0, ncs, eng=None):
            t = streams.tile([128, 2, 4 * HD], MM_DT, tag="wv", bufs=9, name=f"wv{c0}")
            (eng or nc.gpsimd).dma_start(t[:, :ncs, :], wv[:, c0 * 512 : (c0 + ncs) * 512])
            for ci in range(ncs):
                wvt[c0 + ci] = (t, ci)

        def wk01_dma(c0, ncs):
            t = streams.tile([128, 2, 2 * HD], MM_DT, tag="wk01", bufs=8, name=f"wk01_{c0}")
            nc.scalar.dma_start(t[:, :ncs, :], wk01[:, c0 * 256 : (c0 + ncs) * 256])
            for ci in range(ncs):
                wk01t[c0 + ci] = (t, ci)

        def wk23_dma(c0, ncs):
            t = streams.tile([128, 2, 2 * HD], MM_DT, tag="wk23", bufs=8, name=f"wk23_{c0}")
            nc.sync.dma_start(t[:, :ncs, :], wk23[:, c0 * 256 : (c0 + ncs) * 256])
            for ci in range(ncs):
                wk23t[c0 + ci] = (t, ci)

        # The first three cts of x/wv ride the EMPTY low-latency HWDGE rings
        # (first bytes land ~1.5us before the first real matmuls want them);
        # the bulk (cts 3-15) rides the gpsimd SWDGE queue — measured
        # ~1.5-2x the per-queue throughput of a contended HWDGE ring —
        # whose ramp completes while the HWDGE-served cts are consumed.
        # wk01 then follows on scalar, wk23 on sync. Chunk emission is
        # ordered so every lane-collision wait (idx-8) points at a short,
        # long-finished transfer.
        x_dma(0, 1, nc.scalar)
        wv_dma(0, 1, nc.sync)
        x_dma(3, 1)
        wv_dma(3, 1)
        x_dma(1, 1, nc.scalar)
        wv_dma(1, 1, nc.sync)
        x_dma(4, 2)
        wv_dma(4, 2)
        # wrap row 1 (idx 8+): each waits its lane predecessor (idx-8)
        x_dma(2, 1, nc.scalar)
        wv_dma(2, 1, nc.sync)
        x_dma(6, 2)
        wv_dma(6, 2)
        nc.scalar.dma_start(cosT_sb[:], cosT)
        if causal:
            nc.sync.dma_start(maskT_sb[:], maskT)
        else:
            nc.sync.dma_start(
                maskT_sb[:], maskT.rearrange("(st p) t -> p st t", p=128)
            )
        x_dma(8, 2)
        wv_dma(8, 2)
        # row 2
        wk01_dma(0, 2)
        wk23_dma(0, 2)
        x_dma(10, 2)
        wv_dma(10, 2)
        nc.scalar.dma_start(sinT_sb[:], sinT)
        wk23_dma(2, 2)
        x_dma(12, 2)
        wv_dma(12, 2)
        # row 3
        wk01_dma(2, 2)
        wk23_dma(4, 2)
        x_dma(14, 2)
        wv_dma(14, 2)
        wk01_dma(4, 2)
        wk23_dma(6, 2)
        wk01_dma(6, 2)
        wk23_dma(8, 2)

        # wq head-slice prefetch (pre-tiled: one 4KB descriptor/partition)
        wqh_tiles = {}

        def wqh_dma(h, eng=None):
            if h >= NH:
                return
            wqh = streams.tile([128, CT, HD], MM_DT, tag="wqh", bufs=6, name=f"wqh{h}")
            (eng or (nc.scalar if h % 2 == 0 else nc.sync)).dma_start(
                wqh[:], wq[_s(h, 128), :]
            )
            wqh_tiles[h] = wqh

        # row 4
        wk01_dma(8, 2)
        wk23_dma(10, 2)
        wqh_dma(1, nc.sync)
        wk01_dma(10, 2)
        wk23_dma(12, 2)
        wk01_dma(12, 2)
        wk23_dma(14, 2)
        wk01_dma(14, 2)
        # row 5
        wqh_dma(0, nc.scalar)
        wqh_dma(2, nc.scalar)

        # ---- rope helper: psrc (PSUM, [d, t] f32) -> out_slice (SBUF) ----
        # rotate-half via partition-shifted PSUM reads; sinT_sb rows 0:64 are
        # pre-negated on the host, so no rotation matmul is needed.
        def rope(psrc, out_slice, tag, eng=None, copy_eng=None):
            eng = eng or nc.vector
            copy_eng = copy_eng or nc.scalar
            qrot = work.tile([HD, T], MM_DT, tag="trot", name=f"qrot_{tag}")
            cp = (
                copy_eng.copy
                if copy_eng is nc.scalar
                else copy_eng.tensor_copy
            )
            cp(qrot[0:64, :], psrc[64:128, :])
            cp(qrot[64:128, :], psrc[0:64, :])
            tcos = work.tile([HD, T], MM_DT, tag="tcos", name=f"tcos_{tag}")
            eng.tensor_mul(tcos[:], psrc, cosT_sb[:])
            eng.tensor_mul(qrot[:], qrot[:], sinT_sb[:])
            eng.tensor_add(out_slice, tcos[:], qrot[:])

        # PSUM "half" tiles: 2 banks each, double-buffered
        def half_pair(name):
            a = ps.tile([128, 2, 512], F32, tag="sthalf", bufs=2, name=f"{name}a")
            b = ps.tile([128, 2, 512], F32, tag="sthalf", bufs=2, name=f"{name}b")
            return a, b

        # ---- phase A: v and k projections interleaved per ct ----
        vp_a, vp_b = half_pair("vp")
        vps = [vp_a[:, 0, :], vp_a[:, 1, :], vp_b[:, 0, :], vp_b[:, 1, :]]
        # pacing ticks: tiny dummy matmuls at the head of the early ct
        # groups absorb the DMA-ramp shortfall and keep the HAM activity
        # window busy (a >3.4us PE idle re-throttles the clock to 1.2GHz
        # for 10+us). The tick bank is the av bank, idle until k23 starts.
        tickps = ps.tile([HD, T], F32, tag="av", bufs=1, name="tickps")

        def tick(n):
            for _ in range(n):
                mm(
                    tickps[:, :128],
                    warm_sb[:, :128],
                    warm_sb[:, :128],
                    start=True,
                    stop=True,
                )

        _kp_tags = (("qp", 2), ("qp", 2), ("av", 1), ("lsum", 1))
        kps = [
            ps.tile([HD, T], F32, tag=t, bufs=bf, name=f"kp{j}")
            for j, (t, bf) in enumerate(_kp_tags)
        ]

        def vmm(ct):
            t, ci = wvt[ct]
            for i in range(TT):
                mm(
                    vps[i],
                    xT_sb[:, ct, _s(i, 128)],
                    t[:, ci, :],
                    start=(ct == 0),
                    stop=(ct == CT - 1),
                )

        def kmm(ct, js):
            tiles = {0: wk01t, 1: wk01t, 2: wk23t, 3: wk23t}
            for j in js:
                t, ci = tiles[j][ct]
                mm(
                    kps[j][:],
                    t[:, ci, _s(j % 2, HD)],
                    xT_sb[:, ct, :],
                    start=(ct == 0),
                    stop=(ct == CT - 1),
                )

        # k01 lags v by 2 cts early on, then catches up 2-at-a-time so its
        # contraction finishes by the v(12) group — rope-k0/k1 then frees
        # the qp psum banks before the first q-head projection needs them.
        # k23 runs 6 cts behind; its tail (and rope-k2/k3) slides into the
        # phase-B prologue where it has several-us slack and provides PE
        # cover for the rope latency.
        k01_sched = {8: (6, 7), 9: (8, 9), 10: (10, 11), 11: (12, 13), 12: (14, 15)}
        # ticks only BEFORE the first k23 matmul: the tick tile shares the av
        # bank with kps[2], so any tick emitted after k23 starts would
        # serialize k23 behind the last tick (tile-lifetime dependency).
        # From ct4 the pacing comes from real k23 matmuls instead (lag-4 —
        # their wk23/x chunks land early on the dedicated rings).
        tick_sched = {0: 7, 1: 7, 2: 8}
        for ct in range(CT):
            tick(tick_sched.get(ct, 0))
            vmm(ct)
            if 2 <= ct < 8:
                kmm(ct - 2, (0, 1))
            for c in k01_sched.get(ct, ()):
                kmm(c, (0, 1))
            if ct >= 3:
                kmm(ct - 3, (2, 3))
        rope(kps[0][:], kT_sb[:, 0, :], "k0")
        rope(kps[1][:], kT_sb[:, 1, :], "k1")
        for i in range(TT):
            nc.vector.tensor_copy(v_sb[:, i, :], vps[i])

        # ---- phase B: per q head, software pipelined ----
        state = {}

        def stage_a(h):  # projection matmuls into psum
            qp = ps.tile([HD, T], F32, tag="qp", bufs=2, name=f"qp{h}")
            wqh = wqh_tiles.pop(h)
            for ct in range(CT):
                mm(
                    qp[:],
                    wqh[:, ct, :],
                    xT_sb[:, ct, :],
                    start=(ct == 0),
                    stop=(ct == CT - 1),
                )
            wqh_dma(h + 3)
            state[h] = {"qp": qp}

        def stage_b(h):  # rope (straight from psum) -> qT
            qT = work.tile([HD, T], MM_DT, tag="qT", bufs=2, name=f"qT{h}")
            rope(state[h]["qp"][:], qT[:], f"q{h}")
            state[h]["qT"] = qT

        def stage_c1(h):  # scoresT matmuls, exp, mask (per s-tile bank)
            j = h // REP
            st_a, st_b = half_pair(f"sT{h}")
            sts = [st_a[:, 0, :], st_a[:, 1, :], st_b[:, 0, :], st_b[:, 1, :]]
            qT = state[h]["qT"]
            for i in range(TT):
                lo = 128 * i if causal else 0
                mm(
                    sts[i][:, lo:],
                    kT_sb[:, j, _s(i, 128)],
                    qT[:, lo:],
                    start=True,
                    stop=True,
                )
            expm = work.tile([128, TT, T], MM_DT, tag="expm", bufs=2, name=f"expm{h}")
            for i in range(TT):
                lo = 128 * i if causal else 0
                nc.scalar.activation(
                    expm[:, i, lo:], sts[i][:, lo:], AF.Exp, scale=SCALE
                )
                if causal:
                    # only the diagonal 128x128 block is partially masked;
                    # t < lo is never read downstream, t >= lo+128 is fully
                    # visible; the upper-tri pattern is the same for every
                    # diagonal block
                    nc.vector.tensor_mul(
                        expm[:, i, lo : lo + 128],
                        expm[:, i, lo : lo + 128],
                        maskT_sb[:, :],
                    )
                else:
                    nc.vector.tensor_mul(
                        expm[:, i, :], expm[:, i, :], maskT_sb[:, i, :]
                    )
            state[h]["expm"] = expm

        def stage_c2(h):  # AV + normalizer matmuls, reciprocal, scale into aout
            j = h // REP
            expm = state[h]["expm"]
            avp = ps.tile([HD, T], F32, tag="av", bufs=1, name=f"avp{h}")
            for i in range(TT):
                lo = 128 * i if causal else 0
                mm(
                    avp[:, lo:],
                    v_sb[:, i, _s(j, HD)],
                    expm[:, i, lo:],
                    start=(i == 0),
                    stop=(i == TT - 1),
                )
            # normalizer: assemble the column-wise running sum of the s-tiles
            # on the DVE into one contiguous [128, T] tile, then a single
            # N=512 ones-matmul does the partition reduction (gpsimd's
            # partition_all_reduce was tried here: 3.5us/call, pipeline killer)
            lp = ps.tile([128, T], F32, tag="lsum", bufs=1, name=f"lp{h}")
            esum = work.tile([128, T], MM_DT, tag="esum", bufs=2, name=f"esum{h}")
            if causal:
                tmp = work.tile([128, 256], MM_DT, tag="etmp", name=f"etmp{h}")
                tmp2 = work.tile([128, 128], MM_DT, tag="etmp2", name=f"etmp2{h}")
                nc.vector.tensor_copy(esum[:, 0:128], expm[:, 0, 0:128])
                nc.vector.tensor_add(
                    esum[:, 128:256], expm[:, 0, 128:256], expm[:, 1, 128:256]
                )
                nc.vector.tensor_add(tmp[:], expm[:, 0, 256:], expm[:, 1, 256:])
                nc.vector.tensor_add(
                    esum[:, 256:384], tmp[:, 0:128], expm[:, 2, 256:384]
                )
                nc.vector.tensor_add(tmp2[:], tmp[:, 128:], expm[:, 2, 384:])
                nc.vector.tensor_add(esum[:, 384:], tmp2[:], expm[:, 3, 384:])
            else:
                ea = work.tile([128, T], MM_DT, tag="ea", name=f"ea{h}")
                nc.vector.tensor_add(ea[:], expm[:, 0, :], expm[:, 1, :])
                eb = work.tile([128, T], MM_DT, tag="eb", name=f"eb{h}")
                nc.vector.tensor_add(eb[:], expm[:, 2, :], expm[:, 3, :])
                nc.vector.tensor_add(esum[:], ea[:], eb[:])
            mm(lp[:], ones_sb[:], esum[:], start=True, stop=True)
            recip = work.tile([HD, T], F32, tag="recip", name=f"recip{h}")
            nc.vector.reciprocal_approx_fast(recip[:], lp[:HD, :])
            nc.vector.tensor_mul(aout_sb[:, h, :], avp[:], recip[:])
            del state[h]

        # prologue: the k23 projection tail interleaves with the first two
        # q-head projections. The first three k23 groups give the PE cover
        # for rope-k0/k1's psum reads (which free stage_a(0)'s bank);
        # stage_b(0) is emitted BEFORE rope-k2/k3 so rope-q0 isn't queued
        # behind them on the DVE, and rope-k2/k3 runs on the (idle from
        # here) gpsimd engine so neither the DVE nor the scalar exp stream
        # ever waits behind it.
        kmm(13, (2, 3))
        kmm(14, (2, 3))
        kmm(15, (2, 3))
        stage_a(0)
        stage_a(1)
        stage_b(0)
        # rope-k2/k3 directly after rope-q0 in the DVE/scalar queues: kps[2/3]
        # completed before stage_a(0), so these run under the q projections —
        # and stage_c2(0)'s DVE chain no longer queues behind them at it=3
        rope(kps[2][:], kT_sb[:, 2, :], "k2")
        rope(kps[3][:], kT_sb[:, 3, :], "k3")
        for it in range(2, NH + 3):
            if it < NH:
                stage_a(it)
            if 0 <= it - 3 < NH:
                stage_c2(it - 3)
            if it - 1 < NH:
                stage_b(it - 1)
            if 0 <= it - 2 < NH:
                stage_c1(it - 2)

        # ---- phase C: output projection y = aout.T @ wo ----
        # eg order: the qp/av/lsum-bank group first (those banks free during
        # the phase-B tail, so its matmuls can fill phase-B bubbles; the
        # "sthalf" banks only free after the last exp). The last two egs run
        # i-outer so each t-tile's copy + store DMA overlaps the remaining
        # t-tiles' matmuls instead of draining after the final matmul; y is
        # written in MM_DT (fp16) and cast back on the host.
        def yout(eg, i, yslice, final_eg):
            ysb_i = work.tile(
                [128, 512], MM_DT, tag="ysb1", bufs=8, name=f"ysb{eg}_{i}"
            )
            if i % 2 == 0:
                nc.scalar.copy(ysb_i[:], yslice)
            else:
                nc.vector.tensor_copy(ysb_i[:], yslice)
            if final_eg:
                # all HWDGE: a gpsimd store here puts SWDGE's ~1-2us
                # completion latency on the critical teardown path
                eng = (nc.scalar, nc.sync, nc.scalar, nc.sync)[i]
            else:
                eng = nc.scalar if i % 2 == 0 else nc.sync
            eng.dma_start(y[_s(i, 128), _s(eg, 512)], ysb_i[:])

        for pos, eg in enumerate((1, 0, 3, 2)):
            drain = pos >= 2  # i-outer + interleaved store
            if eg % 2 == 0:
                yp_a, yp_b = half_pair(f"yp{eg}")
                yslices = [yp_a[:, 0, :], yp_a[:, 1, :], yp_b[:, 0, :], yp_b[:, 1, :]]
            else:
                yts = [
                    ps.tile([128, 512], F32, tag=t, bufs=bf, name=f"yp{eg}_{i}")
                    for i, (t, bf) in enumerate(
                        (("qp", 2), ("qp", 2), ("av", 1), ("lsum", 1))
                    )
                ]
                yslices = [t[:] for t in yts]
            wots = []
            for fg in range(4):
                wot = streams.tile(
                    [128, 4, 512], MM_DT, tag="wot", bufs=8, name=f"wot{eg}_{fg}"
                )
                (nc.sync if fg % 2 == 0 else nc.scalar).dma_start(
                    wot[:],
                    wo[_s(fg, 512), _s(eg, 512)].rearrange("(c p) e -> p c e", p=128),
                )
                wots.append(wot)
                if not drain:
                    for ci in range(4):
                        ft = fg * 4 + ci
                        for i in range(TT):
                            mm(
                                yslices[i],
                                aout_sb[:, ft, _s(i, 128)],
                                wot[:, ci, :],
                                start=(ft == 0),
                                stop=(ft == CT - 1),
                            )
            if drain:
                final = pos == 3
                for i in range(TT):
                    if final and i == TT - 1:
                        # column-split the very last chain so the first
                        # half's copy + store overlaps the second half's
                        # matmuls, shrinking the post-matmul drain. The two
                        # halves use SEPARATE psum tiles (the qp/av banks are
                        # free by now) — sharing one tile makes the second
                        # half's matmuls wait on the first half's copy
                        # (tile-granular write-after-read).
                        ysb = work.tile(
                            [128, 512], MM_DT, tag="ysb1", bufs=8, name=f"ysbF{i}"
                        )
                        yfin = [
                            ps.tile([128, 512], F32, tag="qp", bufs=2, name="yfinA"),
                            ps.tile([128, 512], F32, tag="av", bufs=1, name="yfinB"),
                        ]
                        for half in range(2):
                            cols = slice(256 * half, 256 * (half + 1))
                            for fg in range(4):
                                for ci in range(4):
                                    ft = fg * 4 + ci
                                    mm(
                                        yfin[half][:, cols],
                                        aout_sb[:, ft, _s(i, 128)],
                                        wots[fg][:, ci, cols],
                                        start=(ft == 0),
                                        stop=(ft == CT - 1),
                                    )
                            if half == 0:
                                nc.scalar.copy(ysb[:, cols], yfin[0][:, cols])
                                nc.scalar.dma_start(
                                    y[_s(i, 128), eg * 512 : eg * 512 + 256],
                                    ysb[:, 0:256],
                                )
                            else:
                                nc.vector.tensor_copy(ysb[:, cols], yfin[1][:, cols])
                                nc.sync.dma_start(
                                    y[_s(i, 128), eg * 512 + 256 : (eg + 1) * 512],
                                    ysb[:, 256:512],
                                )
                    else:
                        for fg in range(4):
                            for ci in range(4):
                                ft = fg * 4 + ci
                                mm(
                                    yslices[i],
                                    aout_sb[:, ft, _s(i, 128)],
                                    wots[fg][:, ci, :],
                                    start=(ft == 0),
                                    stop=(ft == CT - 1),
                                )
                        yout(eg, i, yslices[i], final)
            else:
                for i in range(TT):
                    yout(eg, i, yslices[i], False)


def build(causal=False):
    nc = bacc.Bacc(
        "TRN2",
        target_bir_lowering=False,
        debug=False,
        enable_asserts=False,
        num_devices=N_CORES,
    )
    # x/wq/wk/wv pre-tiled on the host into SBUF layout (one contiguous
    # descriptor per partition per transfer); wk split into kv-head halves
    xT = nc.dram_tensor("xT", [128, CT * T], MM_DT, kind="ExternalInput").ap()
    wq = nc.dram_tensor("wq", [NH * 128, CT * HD], MM_DT, kind="ExternalInput").ap()
    wk01 = nc.dram_tensor("wk01", [128, CT * 2 * HD], MM_DT, kind="ExternalInput").ap()
    wk23 = nc.dram_tensor("wk23", [128, CT * 2 * HD], MM_DT, kind="ExternalInput").ap()
    wv = nc.dram_tensor("wv", [128, CT * 4 * HD], MM_DT, kind="ExternalInput").ap()
    wo = nc.dram_tensor("wo", [C, C], MM_DT, kind="ExternalInput").ap()
    cosT = nc.dram_tensor("cosT", [HD, T], MM_DT, kind="ExternalInput").ap()
    sinT = nc.dram_tensor("sinT", [HD, T], MM_DT, kind="ExternalInput").ap()
    maskT = nc.dram_tensor(
        "maskT", [128, 128] if causal else [T, T], MM_DT, kind="ExternalInput"
    ).ap()
    y = nc.dram_tensor("y", [T, C], MM_DT, kind="ExternalOutput").ap()

    with tile.TileContext(nc) as tc:
        _emit(tc, xT, wq, wk01, wk23, wv, wo, cosT, sinT, maskT, y, causal)
    nc.compile()
    return nc


_NC = {}


def _get_nc(causal):
    if causal not in _NC:
        _NC[causal] = build(causal)
    return _NC[causal]


def _is_causal(mask):
    return bool(np.array_equal(mask, np.tril(np.ones((T, T), dtype=bool))))


def host_tables():
    """cos/sin tables (transposed) and the signed rotate-half matrix."""
    inv = 1.0 / (10000.0 ** (np.arange(0, HD, 2, dtype=np.float32) / HD))
    t = np.arange(T, dtype=np.float32)
    freqs = np.outer(t, inv)  # [T, HD/2]
    emb = np.concatenate([freqs, freqs], axis=-1)  # [T, HD]
    cosT = np.ascontiguousarray(np.cos(emb).T, dtype=np.float32)
    sinT = np.ascontiguousarray(np.sin(emb).T, dtype=np.float32)
    # rotate-half signs baked in: rows d<64 multiply the shifted-down half
    # with a minus sign (q'[d] = q[d]cos - q[d+64]sin for d<64)
    sinT[: HD // 2] *= -1.0
    return cosT, sinT


def make_in_maps(inputs, causal=None):
    x = np.asarray(inputs["x"], dtype=np.float32)
    mask = np.asarray(inputs["mask"]).reshape(T, T)
    if causal is None:
        causal = _is_causal(mask)
    cosT, sinT = host_tables()
    if causal:
        # every diagonal 128x128 block of the causal [s,t] mask is the same
        # upper triangle
        maskT = np.triu(np.ones((128, 128), dtype=MM_NP))
    else:
        maskT = np.ascontiguousarray(mask.T).astype(MM_NP)  # [s, t]
    # pre-tile wq/wk/wv into SBUF layout: dram[p, ct, d] = w[ct*128+p, d]
    # (wq additionally head-major: dram[h*128+p, ct*HD+d] = wq[ct*128+p, h*HD+d])
    wq_f = np.asarray(inputs["wq"], dtype=np.float32)
    wk_f = np.asarray(inputs["wk"], dtype=np.float32)
    wv_f = np.asarray(inputs["wv"], dtype=np.float32)
    wq_t = (
        wq_f.reshape(CT, 128, NH, HD)
        .transpose(2, 1, 0, 3)
        .reshape(NH * 128, CT * HD)
    )
    wk01_t = wk_f[:, : 2 * HD].reshape(CT, 128, 2 * HD).transpose(1, 0, 2).reshape(128, -1)
    wk23_t = wk_f[:, 2 * HD :].reshape(CT, 128, 2 * HD).transpose(1, 0, 2).reshape(128, -1)
    wv_t = wv_f.reshape(CT, 128, 4 * HD).transpose(1, 0, 2).reshape(128, -1)
    shared = {
        "wq": np.ascontiguousarray(wq_t.astype(MM_NP)),
        "wk01": np.ascontiguousarray(wk01_t.astype(MM_NP)),
        "wk23": np.ascontiguousarray(wk23_t.astype(MM_NP)),
        "wv": np.ascontiguousarray(wv_t.astype(MM_NP)),
        "wo": np.ascontiguousarray(np.asarray(inputs["wo"]).astype(MM_NP)),
        "cosT": cosT.astype(MM_NP),
        "sinT": sinT.astype(MM_NP),
        "maskT": maskT,
    }
    return [
        {
            "xT": np.ascontiguousarray(
                x[b].T.reshape(CT, 128, T).transpose(1, 0, 2).reshape(128, CT * T)
            ).astype(MM_NP),
            **shared,
        }
        for b in range(N_CORES)
    ]


def run(inputs, **kw):
    mask = np.asarray(inputs["mask"]).reshape(T, T)
    causal = _is_causal(mask)
    nc = _get_nc(causal)
    in_maps = make_in_maps(inputs, causal)
    res = run_bass_kernel_spmd(nc, in_maps, core_ids=list(range(N_CORES)), **kw)
    out = np.stack([r["y"] for r in res.results], axis=0).astype(np.float32)
    return out, res


def kernel(**inputs) -> np.ndarray:
    out, _ = run(inputs)
    return out
